# revision 15
# baseline (speedup 1.0000x reference)
"""GAT (graph attention) Bass kernel for Trainium2, 8-core SPMD.

Problem (hardcoded shapes): N=4096 nodes, FIN=256, H=8 heads, F=64.
  proj   = (x @ W.T)                         [N, H*F]
  s_src  = sum(proj*a_src, -1), s_tgt likewise
  scores = leaky_relu(s_src[i] + s_tgt[j], 0.2)
  alpha  = softmax(scores + mask, axis=j)
  out    = elu(alpha @ proj + x @ skip_W.T + bias)

Sharding: node-dim (rows i) split 8 ways; every core computes the full
proj locally (cheap) so no collectives are needed.  Per core the hot
loop materializes p[j, i] = exp(leaky(mask_T + s_src[i] + s_tgt[j])) in
fp16 tiles [j=128, i=512] and reduces over j on the TensorEngine with a
ones-column appended to proj to produce the softmax denominator in the
same matmul.
"""

import os
import numpy as np

N = 4096
FIN = 256
H = 8
F = 64
HF = H * F            # 512
NCORES = 8
R = N // NCORES       # 512 rows per core
NB = N // 128         # 32 j-blocks
IC = R // 128         # 4 i-chunks
KC = FIN // 128       # 2 k-chunks

_cache = {}


def _build():
    import os as _os
    ABL = set(_os.environ.get("GAT_ABLATE", "").split(","))
    import concourse.bass as bass
    import concourse.tile as tile
    from concourse import bacc, mybir, masks
    from concourse.alu_op_type import AluOpType as op

    f32 = mybir.dt.float32
    f16 = mybir.dt.float16
    AF = mybir.ActivationFunctionType

    nc = bacc.Bacc("TRN2", target_bir_lowering=False, debug=False,
                   num_devices=NCORES)

    # ---- DRAM I/O ----
    x_d = nc.dram_tensor("x", [N, FIN], f32, kind="ExternalInput")
    xblk_d = nc.dram_tensor("xblk", [R, FIN], f32, kind="ExternalInput")
    mask_d = nc.dram_tensor("mask", [R, N], f32, kind="ExternalInput")
    W_d = nc.dram_tensor("w", [HF, FIN], f32, kind="ExternalInput")
    sW_d = nc.dram_tensor("skip_w", [HF, FIN], f32, kind="ExternalInput")
    bias_d = nc.dram_tensor("bias", [HF], f32, kind="ExternalInput")
    asrc_d = nc.dram_tensor("a_src", [HF], f32, kind="ExternalInput")
    atgt_d = nc.dram_tensor("a_tgt", [HF], f32, kind="ExternalInput")
    out_d = nc.dram_tensor("out", [R, HF], f32, kind="ExternalOutput")

    # fp16 DRAM staging for the DMA-transpose path (xbar needs 2-byte dtype)
    mask16_d = nc.dram_tensor("mask16", [R, N], f16, kind="Internal")
    xs16_d = nc.dram_tensor("xs16", [N, FIN], f16, kind="Internal")
    xb16_d = nc.dram_tensor("xb16", [R, FIN], f16, kind="Internal")
    Ws16_d = nc.dram_tensor("ws16", [HF, FIN], f16, kind="Internal")
    sWs16_d = nc.dram_tensor("sws16", [HF, FIN], f16, kind="Internal")
    zscr_d = nc.dram_tensor("zscr", [H, R], f32, kind="Internal")

    from contextlib import ExitStack
    with tile.TileContext(nc) as tc, \
         tc.tile_pool(name="persist", bufs=1) as pp:
        prep_ctx = ExitStack()
        prep = prep_ctx.enter_context(tc.tile_pool(name="prep", bufs=1))

        # ================= phase 0: casts + transposes (DMA) ============
        # f32 -> fp16 casts (SWDGE).  DRAM->DRAM.
        nc.gpsimd.dma_start(out=mask16_d.ap(), in_=mask_d.ap())
        nc.gpsimd.dma_start(out=xs16_d.ap(), in_=x_d.ap())
        nc.gpsimd.dma_start(out=xb16_d.ap(), in_=xblk_d.ap())
        nc.gpsimd.dma_start(out=Ws16_d.ap(), in_=W_d.ap())
        nc.gpsimd.dma_start(out=sWs16_d.ap(), in_=sW_d.ap())

        # transposed fp16 views via xbar DMA-transpose (HWDGE)
        maskT = pp.tile([128, NB, R], f16)      # [j_lo, jb, i]
        if "maskt" not in ABL:
            for jb in range(NB):
                nc.sync.dma_start(out=maskT[:, jb, :],
                                  in_=mask16_d.ap()[:, bass.ts(jb, 128)],
                                  transpose=True)
        else:
            nc.sync.dma_start(out=maskT[:, 0, :],
                              in_=mask16_d.ap()[:, bass.ts(0, 128)],
                              transpose=True)
        xT = prep.tile([128, KC, N], f16)         # [k_lo, kc, n]
        for kc in range(KC):
            nc.sync.dma_start(out=xT[:, kc, :],
                              in_=xs16_d.ap()[:, bass.ts(kc, 128)],
                              transpose=True)
        xbT = prep.tile([128, KC, R], f16)        # [k_lo, kc, i] (own rows)
        for kc in range(KC):
            nc.sync.dma_start(out=xbT[:, kc, :],
                              in_=xb16_d.ap()[:, bass.ts(kc, 128)],
                              transpose=True)
        WT = prep.tile([128, KC, HF], f16)        # [k_lo, kc, hf]
        sWT = prep.tile([128, KC, HF], f16)
        for kc in range(KC):
            nc.sync.dma_start(out=WT[:, kc, :],
                              in_=Ws16_d.ap()[:, bass.ts(kc, 128)],
                              transpose=True)
            nc.sync.dma_start(out=sWT[:, kc, :],
                              in_=sWs16_d.ap()[:, bass.ts(kc, 128)],
                              transpose=True)

        # untransposed small loads
        Wsb = prep.tile([128, 4, FIN], f16)       # [hf_lo, hfc, k]
        nc.sync.dma_start(out=Wsb[:],
                          in_=Ws16_d.ap().rearrange("(c p) k -> p c k", p=128))
        acol_src = prep.tile([128, 4], f32)       # a_src as [hf_lo, hfc]
        acol_tgt = prep.tile([128, 4], f32)
        nc.sync.dma_start(out=acol_src[:],
                          in_=asrc_d.ap().rearrange("(c p) -> p c", p=128))
        nc.sync.dma_start(out=acol_tgt[:],
                          in_=atgt_d.ap().rearrange("(c p) -> p c", p=128))
        bias16 = prep.tile([1, HF], f16)
        nc.gpsimd.dma_start(out=bias16[:], in_=bias_d.ap().unsqueeze(0))

        # constants
        ones_row = prep.tile([1, 128], f16)
        nc.vector.memset(ones_row[:], 1.0)
        hsel = prep.tile([128, 2], f16)           # head-selector 0/1 columns
        nc.vector.memset(hsel[:], 0.0)
        nc.vector.memset(hsel[0:64, 0:1], 1.0)
        nc.vector.memset(hsel[64:128, 1:2], 1.0)
        ident = pp.tile([128, 128], f32)
        masks.make_identity(nc, ident[:])

        # ================= phase 1: PE preprocessing ====================
        with tc.tile_pool(name="ps_big", bufs=2, space="PSUM") as psb, \
             tc.tile_pool(name="ps_small", bufs=2, space="PSUM") as pss:

            # wsc_{src,tgt}[hf, k] = W[hf, k] * a[hf]
            wsc_src = prep.tile([128, 4, FIN], f16)
            wsc_tgt = prep.tile([128, 4, FIN], f16)
            for c in range(4):
                nc.vector.tensor_scalar_mul(wsc_src[:, c, :], Wsb[:, c, :],
                                            acol_src[:, c:c + 1])
                nc.vector.tensor_scalar_mul(wsc_tgt[:, c, :], Wsb[:, c, :],
                                            acol_tgt[:, c:c + 1])

            # ws_{src,tgt}[k, h] = sum_f W[(h,f), k] * a[h, f]
            ws_src = pp.tile([128, KC, H], f16)
            ws_tgt = pp.tile([128, KC, H], f16)
            for kc in range(KC):
                for (wsc, ws) in ((wsc_src, ws_src), (wsc_tgt, ws_tgt)):
                    pw = pss.tile([128, H], f32, tag="small")
                    for c in range(4):
                        nc.tensor.matmul(pw[:, 2 * c:2 * c + 2],
                                         wsc[:, c, bass.ts(kc, 128)],
                                         hsel[:],
                                         start=True, stop=True)
                    nc.vector.tensor_copy(ws[:, kc, :], pw[:])

            # projE[n, h, 0:F] = proj, projE[n, h, F] = 1.0
            projE = pp.tile([128, NB, H, F + 1], f16)
            for nb in range(NB):
                ps = psb.tile([128, HF], f32, tag="big")
                for kc in range(KC):
                    nc.tensor.matmul(ps[:], xT[:, kc, bass.ts(nb, 128)],
                                     WT[:, kc, :],
                                     start=(kc == 0), stop=(kc == KC - 1))
                ps_hf = ps[:].rearrange("p (h f) -> p h f", f=F)
                if nb % 2 == 0:
                    nc.scalar.activation(projE[:, nb, :, 0:F], ps_hf, AF.Copy)
                else:
                    nc.vector.tensor_copy(projE[:, nb, :, 0:F], ps_hf)
            nc.vector.memset(projE[:, :, :, F:F + 1], 1.0)

            # s_tgt[n, h] for all n (per-partition operand of the TS pass)
            s_tgt_nh = pp.tile([128, NB, H], f32)
            for nb in range(NB):
                pt = pss.tile([128, H], f32, tag="small")
                for kc in range(KC):
                    nc.tensor.matmul(pt[:], xT[:, kc, bass.ts(nb, 128)],
                                     ws_tgt[:, kc, :],
                                     start=(kc == 0), stop=(kc == KC - 1))
                nc.vector.tensor_copy(s_tgt_nh[:, nb, :], pt[:])

            # s_src rows for the core's own i: [h, i]
            s_src_sb = prep.tile([H, R], f16)
            pss2 = psb.tile([H, R], f32, tag="big")
            for kc in range(KC):
                nc.tensor.matmul(pss2[:], ws_src[:, kc, :], xbT[:, kc, :],
                                 start=(kc == 0), stop=(kc == KC - 1))
            nc.vector.tensor_copy(s_src_sb[:], pss2[:])

            # broadcast s_src rows across partitions: sbc[h][j_lo, i]
            # sel8[:, h, :] is an [8, 128] selector picking row h of s_src_sb
            # sel8[h', h, m] = 1 if h' == h else 0, via affine iota compare
            sel8 = prep.tile([8, H, 128], f16)
            nc.gpsimd.memset(sel8[:], 0.0)
            nc.gpsimd.affine_select(
                out=sel8[:], in_=sel8[:],
                compare_op=mybir.AluOpType.not_equal,
                fill=1.0, base=0,
                # iota = h' * 1 + h * (-1) + m * 0; != 0 -> keep 0, == 0 -> 1
                pattern=[[-1, H], [0, 128]],
                channel_multiplier=1)
            sbc = pp.tile([128, H, R], f16)
            for h in range(H):
                pb = psb.tile([128, R], f32, tag="big")
                nc.tensor.matmul(pb[:], sel8[:, h, :], s_src_sb[:],
                                 start=True, stop=True)
                nc.vector.tensor_copy(sbc[:, h, :], pb[:])

            # skip projection + bias (bias folded as a rank-1 accumulate)
            skipb = pp.tile([128, IC, HF], f32)
            for ic in range(IC):
                pk = psb.tile([128, HF], f32, tag="big")
                for kc in range(KC):
                    nc.tensor.matmul(pk[:], xbT[:, kc, bass.ts(ic, 128)],
                                     sWT[:, kc, :],
                                     start=(kc == 0), stop=False)
                nc.tensor.matmul(pk[:], ones_row[:], bias16[:],
                                 start=False, stop=True)
                nc.vector.tensor_copy(skipb[:, ic, :], pk[:])

        # ================= phase 2: attention main loop =================
        prep_ctx.close()
        oT = pp.tile([F + 1, H, R], f32)        # [f(+Z), h, i]
        with tc.tile_pool(name="ps_agg", bufs=3, space="PSUM") as psa, \
             tc.tile_pool(name="hbuf", bufs=2) as hpool, \
             tc.tile_pool(name="fin", bufs=2) as fpool:
            for h in range(H):
                v = hpool.tile([128, NB, R], f16, tag="v")
                # v = maskT + s_tgt[j]  (per-partition scalar, per j-block)
                if "ts" not in ABL:
                    for jb in range(NB):
                        nc.vector.tensor_scalar_add(v[:, jb, :], maskT[:, jb, :],
                                                    s_tgt_nh[:, jb, h:h + 1])
                else:
                    nc.vector.tensor_copy(v[:, 0, :], maskT[:, 0, :])
                # v += s_src[i]  (one merged TT, broadcast over jb)
                if "tt" not in ABL:
                    nc.vector.tensor_add(
                        v[:], v[:],
                        sbc[:, h:h + 1, :].broadcast_to([128, NB, R]))
                # p = exp(leaky_relu(v))
                if "act" not in ABL:
                    nc.scalar.activation(v[:], v[:], AF.Prelu, alpha=0.2)
                    nc.scalar.activation(v[:], v[:], AF.Exp)
                # aggregate: psum[f, i] += projE[:, jb, h].T @ p[:, jb]
                pa = psa.tile([128, R], f32, tag="agg")
                if "agg" not in ABL:
                    for jb in range(NB):
                        nc.tensor.matmul(pa[0:F + 1, :], projE[:, jb, h, :],
                                         v[:, jb, :],
                                         start=(jb == 0), stop=(jb == NB - 1))
                else:
                    nc.tensor.matmul(pa[0:F + 1, :], projE[:, 0, h, :],
                                     v[:, 0, :], start=True, stop=True)
                nc.vector.tensor_copy(oT[:, h, :], pa[0:F + 1, :])

            # ============= phase 3: normalize, skip, ELU ================
            # reciprocal of row sums, in place on the Z row of oT
            nc.vector.reciprocal(oT[F:F + 1, :, :], oT[F:F + 1, :, :])
            # shuffle recZ into [i_lo, ic, h] layout (tiny partition-scatter)
            recZT = pp.tile([128, IC, H], f32)
            nc.gpsimd.dma_start(out=zscr_d.ap(), in_=oT[F:F + 1, :, :])
            for ic in range(IC):
                nc.gpsimd.dma_start(
                    out=recZT[:, ic, :],
                    in_=zscr_d.ap()[:, bass.ts(ic, 128)].rearrange("h p -> p h"))

            out_sb = pp.tile([128, IC, HF], f32)
            for ic in range(IC):
                pT = psa.tile([128, HF], f32, tag="agg")
                for h in range(H):
                    nc.tensor.transpose(pT[:, bass.ts(h, F)],
                                        oT[0:F, h, bass.ts(ic, 128)],
                                        ident[0:F, 0:F])
                y = fpool.tile([128, H, F], f32, tag="y")
                nc.vector.tensor_mul(
                    y[:], pT[:].rearrange("p (h f) -> p h f", f=F),
                    recZT[:, ic, :].unsqueeze(2).broadcast_to([128, H, F]))
                nc.vector.tensor_add(
                    y[:], y[:],
                    skipb[:, ic, :].rearrange("p (h f) -> p h f", f=F))
                # elu(y) = max(y, 0) + min(exp(y) - 1, 0)
                q = fpool.tile([128, H, F], f32, tag="q")
                nc.scalar.activation(q[:], y[:], AF.Exp)
                nc.vector.tensor_scalar(q[:], q[:], 1.0, 0.0,
                                        op.subtract, op.min)
                nc.vector.tensor_scalar(y[:], y[:], 0.0, None, op.max)
                nc.vector.tensor_add(
                    out_sb[:, ic, :].rearrange("p (h f) -> p h f", f=F),
                    y[:], q[:])
            nc.sync.dma_start(
                out=out_d.ap().rearrange("(c p) f -> p c f", p=128),
                in_=out_sb[:])

    nc.compile()
    return nc


def _get_nc():
    if "nc" not in _cache:
        _cache["nc"] = _build()
    return _cache["nc"]


def kernel(x, connectivity_mask, W, a_src, a_tgt, skip_W, bias):
    from concourse.bass_utils import run_bass_kernel_spmd

    x = np.ascontiguousarray(np.asarray(x, dtype=np.float32))
    cm = np.ascontiguousarray(np.asarray(connectivity_mask, dtype=np.float32))
    W = np.ascontiguousarray(np.asarray(W, dtype=np.float32))
    sW = np.ascontiguousarray(np.asarray(skip_W, dtype=np.float32))
    b = np.ascontiguousarray(np.asarray(bias, dtype=np.float32))
    asrc = np.ascontiguousarray(np.asarray(a_src, dtype=np.float32).reshape(HF))
    atgt = np.ascontiguousarray(np.asarray(a_tgt, dtype=np.float32).reshape(HF))

    in_maps = []
    for c in range(NCORES):
        in_maps.append({
            "x": x,
            "xblk": np.ascontiguousarray(x[c * R:(c + 1) * R]),
            "mask": np.ascontiguousarray(cm[c * R:(c + 1) * R]),
            "w": W,
            "skip_w": sW,
            "bias": b,
            "a_src": asrc,
            "a_tgt": atgt,
        })

    nc = _get_nc()
    res = run_bass_kernel_spmd(nc, in_maps, core_ids=list(range(NCORES)))
    return np.concatenate([r["out"] for r in res.results], axis=0)


# revision 16
# speedup vs baseline: 1.0899x; 1.0899x over previous
"""GAT (graph attention) Bass kernel for Trainium2, 8-core SPMD.

Problem (hardcoded shapes): N=4096 nodes, FIN=256, H=8 heads, F=64.
  proj   = (x @ W.T)                         [N, H*F]
  s_src  = sum(proj*a_src, -1), s_tgt likewise
  scores = leaky_relu(s_src[i] + s_tgt[j], 0.2)
  alpha  = softmax(scores + mask, axis=j)
  out    = elu(alpha @ proj + x @ skip_W.T + bias)

Sharding: node-dim (rows i) split 8 ways; every core computes the full
proj locally (cheap) so no collectives are needed.  Per core the hot
loop materializes p[j, i] = exp(leaky(mask_T + s_src[i] + s_tgt[j])) in
fp16 tiles [j=128, i=512] and reduces over j on the TensorEngine with a
ones-column appended to proj to produce the softmax denominator in the
same matmul.
"""

import os
import numpy as np

N = 4096
FIN = 256
H = 8
F = 64
HF = H * F            # 512
NCORES = 8
R = N // NCORES       # 512 rows per core
NB = N // 128         # 32 j-blocks
IC = R // 128         # 4 i-chunks
KC = FIN // 128       # 2 k-chunks

_cache = {}


def _build():
    import os as _os
    ABL = set(_os.environ.get("GAT_ABLATE", "").split(","))
    import concourse.bass as bass
    import concourse.tile as tile
    from concourse import bacc, mybir, masks
    from concourse.alu_op_type import AluOpType as op

    f32 = mybir.dt.float32
    f16 = mybir.dt.float16
    AF = mybir.ActivationFunctionType

    nc = bacc.Bacc("TRN2", target_bir_lowering=False, debug=False,
                   num_devices=NCORES)

    # ---- DRAM I/O ----
    x_d = nc.dram_tensor("x", [N, FIN], f32, kind="ExternalInput")
    xblk_d = nc.dram_tensor("xblk", [R, FIN], f32, kind="ExternalInput")
    mask_d = nc.dram_tensor("mask", [R, N], f32, kind="ExternalInput")
    W_d = nc.dram_tensor("w", [HF, FIN], f32, kind="ExternalInput")
    sW_d = nc.dram_tensor("skip_w", [HF, FIN], f32, kind="ExternalInput")
    bias_d = nc.dram_tensor("bias", [HF], f32, kind="ExternalInput")
    asrc_d = nc.dram_tensor("a_src", [HF], f32, kind="ExternalInput")
    atgt_d = nc.dram_tensor("a_tgt", [HF], f32, kind="ExternalInput")
    out_d = nc.dram_tensor("out", [R, HF], f32, kind="ExternalOutput")

    # fp16 DRAM staging for the DMA-transpose path (xbar needs 2-byte dtype)
    mask16_d = nc.dram_tensor("mask16", [R, N], f16, kind="Internal")
    xs16_d = nc.dram_tensor("xs16", [N, FIN], f16, kind="Internal")
    xb16_d = nc.dram_tensor("xb16", [R, FIN], f16, kind="Internal")
    Ws16_d = nc.dram_tensor("ws16", [HF, FIN], f16, kind="Internal")
    sWs16_d = nc.dram_tensor("sws16", [HF, FIN], f16, kind="Internal")
    zscr_d = nc.dram_tensor("zscr", [H, R], f32, kind="Internal")

    from contextlib import ExitStack
    with tile.TileContext(nc) as tc, \
         tc.tile_pool(name="persist", bufs=1) as pp:
        prep_ctx = ExitStack()
        prep = prep_ctx.enter_context(tc.tile_pool(name="prep", bufs=1))

        # ================= phase 0: casts + transposes (DMA) ============
        # f32 -> fp16 casts (SWDGE, DRAM->DRAM).  Small tensors first so
        # the s_src/s_tgt/proj chain isn't stuck behind the 12MB mask cast;
        # mask cast is chunked so its transposes can start early.
        nc.gpsimd.dma_start(out=xs16_d.ap(), in_=x_d.ap())
        nc.gpsimd.dma_start(out=xb16_d.ap(), in_=xblk_d.ap())
        nc.gpsimd.dma_start(out=Ws16_d.ap(), in_=W_d.ap())
        nc.gpsimd.dma_start(out=sWs16_d.ap(), in_=sW_d.ap())
        MCH = 4                                 # mask cast chunks (of 8 jb)
        for mc in range(MCH):
            nc.gpsimd.dma_start(
                out=mask16_d.ap()[:, bass.ts(mc, N // MCH)],
                in_=mask_d.ap()[:, bass.ts(mc, N // MCH)])

        # transposed fp16 views via xbar DMA-transpose (HWDGE).  Small
        # transposes go on the scalar HWDGE queue (ACT is idle in prep);
        # the 32 mask transposes alternate between both queues.
        xT = prep.tile([128, KC, N], f16)         # [k_lo, kc, n]
        xbT = prep.tile([128, KC, R], f16)        # [k_lo, kc, i] (own rows)
        WT = prep.tile([128, KC, HF], f16)        # [k_lo, kc, hf]
        sWT = prep.tile([128, KC, HF], f16)
        for kc in range(KC):
            nc.scalar.dma_start(out=xT[:, kc, :],
                                in_=xs16_d.ap()[:, bass.ts(kc, 128)],
                                transpose=True)
            nc.scalar.dma_start(out=xbT[:, kc, :],
                                in_=xb16_d.ap()[:, bass.ts(kc, 128)],
                                transpose=True)
            nc.scalar.dma_start(out=WT[:, kc, :],
                                in_=Ws16_d.ap()[:, bass.ts(kc, 128)],
                                transpose=True)
            nc.scalar.dma_start(out=sWT[:, kc, :],
                                in_=sWs16_d.ap()[:, bass.ts(kc, 128)],
                                transpose=True)
        maskT = pp.tile([128, NB, R], f16)      # [j_lo, jb, i]
        njb = NB if "maskt" not in ABL else 1
        for jb in range(njb):
            eng = nc.sync if jb % 2 == 0 else nc.scalar
            eng.dma_start(out=maskT[:, jb, :],
                          in_=mask16_d.ap()[:, bass.ts(jb, 128)],
                          transpose=True)

        # untransposed small loads
        Wsb = prep.tile([128, 4, FIN], f16)       # [hf_lo, hfc, k]
        nc.sync.dma_start(out=Wsb[:],
                          in_=Ws16_d.ap().rearrange("(c p) k -> p c k", p=128))
        acol_src = prep.tile([128, 4], f32)       # a_src as [hf_lo, hfc]
        acol_tgt = prep.tile([128, 4], f32)
        nc.sync.dma_start(out=acol_src[:],
                          in_=asrc_d.ap().rearrange("(c p) -> p c", p=128))
        nc.sync.dma_start(out=acol_tgt[:],
                          in_=atgt_d.ap().rearrange("(c p) -> p c", p=128))
        bias16 = prep.tile([1, HF], f16)
        nc.gpsimd.dma_start(out=bias16[:], in_=bias_d.ap().unsqueeze(0))

        # constants
        ones_row = prep.tile([1, 128], f16)
        nc.vector.memset(ones_row[:], 1.0)
        hsel = prep.tile([128, 2], f16)           # head-selector 0/1 columns
        nc.vector.memset(hsel[:], 0.0)
        nc.vector.memset(hsel[0:64, 0:1], 1.0)
        nc.vector.memset(hsel[64:128, 1:2], 1.0)
        ident = pp.tile([128, 128], f32)
        masks.make_identity(nc, ident[:])

        # ================= phase 1: PE preprocessing ====================
        with tc.tile_pool(name="ps_big", bufs=2, space="PSUM") as psb, \
             tc.tile_pool(name="ps_small", bufs=2, space="PSUM") as pss:

            # wsc_{src,tgt}[hf, k] = W[hf, k] * a[hf]
            wsc_src = prep.tile([128, 4, FIN], f16)
            wsc_tgt = prep.tile([128, 4, FIN], f16)
            for c in range(4):
                nc.vector.tensor_scalar_mul(wsc_src[:, c, :], Wsb[:, c, :],
                                            acol_src[:, c:c + 1])
                nc.vector.tensor_scalar_mul(wsc_tgt[:, c, :], Wsb[:, c, :],
                                            acol_tgt[:, c:c + 1])

            # ws_{src,tgt}[k, h] = sum_f W[(h,f), k] * a[h, f]
            ws_src = pp.tile([128, KC, H], f16)
            ws_tgt = pp.tile([128, KC, H], f16)
            for kc in range(KC):
                for (wsc, ws) in ((wsc_src, ws_src), (wsc_tgt, ws_tgt)):
                    pw = pss.tile([128, H], f32, tag="small")
                    for c in range(4):
                        nc.tensor.matmul(pw[:, 2 * c:2 * c + 2],
                                         wsc[:, c, bass.ts(kc, 128)],
                                         hsel[:],
                                         start=True, stop=True)
                    nc.vector.tensor_copy(ws[:, kc, :], pw[:])

            # projE[n, h, 0:F] = proj, projE[n, h, F] = 1.0
            projE = pp.tile([128, NB, H, F + 1], f16)
            for nb in range(NB):
                ps = psb.tile([128, HF], f32, tag="big")
                for kc in range(KC):
                    nc.tensor.matmul(ps[:], xT[:, kc, bass.ts(nb, 128)],
                                     WT[:, kc, :],
                                     start=(kc == 0), stop=(kc == KC - 1))
                ps_hf = ps[:].rearrange("p (h f) -> p h f", f=F)
                if nb % 2 == 0:
                    nc.scalar.activation(projE[:, nb, :, 0:F], ps_hf, AF.Copy)
                else:
                    nc.vector.tensor_copy(projE[:, nb, :, 0:F], ps_hf)
            nc.vector.memset(projE[:, :, :, F:F + 1], 1.0)

            # s_tgt[n, h] for all n (per-partition operand of the TS pass)
            s_tgt_nh = pp.tile([128, NB, H], f32)
            for nb in range(NB):
                pt = pss.tile([128, H], f32, tag="small")
                for kc in range(KC):
                    nc.tensor.matmul(pt[:], xT[:, kc, bass.ts(nb, 128)],
                                     ws_tgt[:, kc, :],
                                     start=(kc == 0), stop=(kc == KC - 1))
                nc.vector.tensor_copy(s_tgt_nh[:, nb, :], pt[:])

            # s_src rows for the core's own i: [h, i]
            s_src_sb = prep.tile([H, R], f16)
            pss2 = psb.tile([H, R], f32, tag="big")
            for kc in range(KC):
                nc.tensor.matmul(pss2[:], ws_src[:, kc, :], xbT[:, kc, :],
                                 start=(kc == 0), stop=(kc == KC - 1))
            nc.vector.tensor_copy(s_src_sb[:], pss2[:])

            # broadcast s_src rows across partitions: sbc[h][j_lo, i]
            # sel8[:, h, :] is an [8, 128] selector picking row h of s_src_sb
            # sel8[h', h, m] = 1 if h' == h else 0, via affine iota compare
            sel8 = prep.tile([8, H, 128], f16)
            nc.gpsimd.memset(sel8[:], 0.0)
            nc.gpsimd.affine_select(
                out=sel8[:], in_=sel8[:],
                compare_op=mybir.AluOpType.not_equal,
                fill=1.0, base=0,
                # iota = h' * 1 + h * (-1) + m * 0; != 0 -> keep 0, == 0 -> 1
                pattern=[[-1, H], [0, 128]],
                channel_multiplier=1)
            sbc = pp.tile([128, H, R], f16)
            for h in range(H):
                pb = psb.tile([128, R], f32, tag="big")
                nc.tensor.matmul(pb[:], sel8[:, h, :], s_src_sb[:],
                                 start=True, stop=True)
                nc.vector.tensor_copy(sbc[:, h, :], pb[:])

            # skip projection + bias (bias folded as a rank-1 accumulate)
            skipb = pp.tile([128, IC, HF], f32)
            for ic in range(IC):
                pk = psb.tile([128, HF], f32, tag="big")
                for kc in range(KC):
                    nc.tensor.matmul(pk[:], xbT[:, kc, bass.ts(ic, 128)],
                                     sWT[:, kc, :],
                                     start=(kc == 0), stop=False)
                nc.tensor.matmul(pk[:], ones_row[:], bias16[:],
                                 start=False, stop=True)
                nc.vector.tensor_copy(skipb[:, ic, :], pk[:])

        # ================= phase 2: attention main loop =================
        prep_ctx.close()
        oT = pp.tile([F + 1, H, R], f32)        # [f(+Z), h, i]
        with tc.tile_pool(name="ps_agg", bufs=3, space="PSUM") as psa, \
             tc.tile_pool(name="hbuf", bufs=2) as hpool, \
             tc.tile_pool(name="fin", bufs=2) as fpool:
            for h in range(H):
                v = hpool.tile([128, NB, R], f16, tag="v")
                # v = maskT + s_tgt[j]  (per-partition scalar, per j-block)
                if "ts" not in ABL:
                    for jb in range(NB):
                        nc.vector.tensor_scalar_add(v[:, jb, :], maskT[:, jb, :],
                                                    s_tgt_nh[:, jb, h:h + 1])
                else:
                    nc.vector.tensor_copy(v[:, 0, :], maskT[:, 0, :])
                # v += s_src[i]  (one merged TT, broadcast over jb)
                if "tt" not in ABL:
                    nc.vector.tensor_add(
                        v[:], v[:],
                        sbc[:, h:h + 1, :].broadcast_to([128, NB, R]))
                # p = exp(leaky_relu(v))
                if "act" not in ABL:
                    nc.scalar.activation(v[:], v[:], AF.Prelu, alpha=0.2)
                    nc.scalar.activation(v[:], v[:], AF.Exp)
                # aggregate: psum[f, i] += projE[:, jb, h].T @ p[:, jb]
                pa = psa.tile([128, R], f32, tag="agg")
                if "agg" not in ABL:
                    for jb in range(NB):
                        nc.tensor.matmul(pa[0:F + 1, :], projE[:, jb, h, :],
                                         v[:, jb, :],
                                         start=(jb == 0), stop=(jb == NB - 1))
                else:
                    nc.tensor.matmul(pa[0:F + 1, :], projE[:, 0, h, :],
                                     v[:, 0, :], start=True, stop=True)
                nc.vector.tensor_copy(oT[:, h, :], pa[0:F + 1, :])

            # ============= phase 3: normalize, skip, ELU ================
            # shuffle Z into [i_lo, ic, h] layout (tiny partition-scatter),
            # then reciprocal across all 128 partitions (fast)
            recZT = pp.tile([128, IC, H], f32)
            nc.gpsimd.dma_start(out=zscr_d.ap(), in_=oT[F:F + 1, :, :])
            for ic in range(IC):
                nc.gpsimd.dma_start(
                    out=recZT[:, ic, :],
                    in_=zscr_d.ap()[:, bass.ts(ic, 128)].rearrange("h p -> p h"))
            nc.vector.reciprocal(recZT[:], recZT[:])

            out_sb = pp.tile([128, IC, HF], f32)
            for ic in range(IC):
                pT = psa.tile([128, HF], f32, tag="agg")
                for h in range(H):
                    nc.tensor.transpose(pT[:, bass.ts(h, F)],
                                        oT[0:F, h, bass.ts(ic, 128)],
                                        ident[0:F, 0:F])
                y = fpool.tile([128, H, F], f32, tag="y")
                nc.vector.tensor_mul(
                    y[:], pT[:].rearrange("p (h f) -> p h f", f=F),
                    recZT[:, ic, :].unsqueeze(2).broadcast_to([128, H, F]))
                nc.vector.tensor_add(
                    y[:], y[:],
                    skipb[:, ic, :].rearrange("p (h f) -> p h f", f=F))
                # elu(y) = max(y, 0) + min(exp(y) - 1, 0)
                q = fpool.tile([128, H, F], f32, tag="q")
                nc.scalar.activation(q[:], y[:], AF.Exp)
                nc.vector.tensor_scalar(q[:], q[:], 1.0, 0.0,
                                        op.subtract, op.min)
                nc.vector.tensor_scalar(y[:], y[:], 0.0, None, op.max)
                nc.vector.tensor_add(
                    out_sb[:, ic, :].rearrange("p (h f) -> p h f", f=F),
                    y[:], q[:])
            nc.sync.dma_start(
                out=out_d.ap().rearrange("(c p) f -> p c f", p=128),
                in_=out_sb[:])

    nc.compile()
    return nc


def _get_nc():
    if "nc" not in _cache:
        _cache["nc"] = _build()
    return _cache["nc"]


def kernel(x, connectivity_mask, W, a_src, a_tgt, skip_W, bias):
    from concourse.bass_utils import run_bass_kernel_spmd

    x = np.ascontiguousarray(np.asarray(x, dtype=np.float32))
    cm = np.ascontiguousarray(np.asarray(connectivity_mask, dtype=np.float32))
    W = np.ascontiguousarray(np.asarray(W, dtype=np.float32))
    sW = np.ascontiguousarray(np.asarray(skip_W, dtype=np.float32))
    b = np.ascontiguousarray(np.asarray(bias, dtype=np.float32))
    asrc = np.ascontiguousarray(np.asarray(a_src, dtype=np.float32).reshape(HF))
    atgt = np.ascontiguousarray(np.asarray(a_tgt, dtype=np.float32).reshape(HF))

    in_maps = []
    for c in range(NCORES):
        in_maps.append({
            "x": x,
            "xblk": np.ascontiguousarray(x[c * R:(c + 1) * R]),
            "mask": np.ascontiguousarray(cm[c * R:(c + 1) * R]),
            "w": W,
            "skip_w": sW,
            "bias": b,
            "a_src": asrc,
            "a_tgt": atgt,
        })

    nc = _get_nc()
    res = run_bass_kernel_spmd(nc, in_maps, core_ids=list(range(NCORES)))
    return np.concatenate([r["out"] for r in res.results], axis=0)


# revision 18
# speedup vs baseline: 1.2738x; 1.1687x over previous
"""GAT (graph attention) Bass kernel for Trainium2, 8-core SPMD.

Problem (hardcoded shapes): N=4096 nodes, FIN=256, H=8 heads, F=64.
  proj   = (x @ W.T)                         [N, H*F]
  s_src  = sum(proj*a_src, -1), s_tgt likewise
  scores = leaky_relu(s_src[i] + s_tgt[j], 0.2)
  alpha  = softmax(scores + mask, axis=j)
  out    = elu(alpha @ proj + x @ skip_W.T + bias)

Sharding: node-dim (rows i) split 8 ways; every core computes the full
proj locally (cheap) so no collectives are needed.  Per core the hot
loop materializes p[j, i] = exp(leaky(mask_T + s_src[i] + s_tgt[j])) in
fp16 tiles [j=128, i=512] and reduces over j on the TensorEngine with a
ones-column appended to proj to produce the softmax denominator in the
same matmul.
"""

import os
import numpy as np

N = 4096
FIN = 256
H = 8
F = 64
HF = H * F            # 512
NCORES = 8
R = N // NCORES       # 512 rows per core
NB = N // 128         # 32 j-blocks
IC = R // 128         # 4 i-chunks
KC = FIN // 128       # 2 k-chunks

_cache = {}


def _build():
    import os as _os
    ABL = set(_os.environ.get("GAT_ABLATE", "").split(","))
    import concourse.bass as bass
    import concourse.tile as tile
    from concourse import bacc, mybir, masks
    from concourse.alu_op_type import AluOpType as op

    f32 = mybir.dt.float32
    f16 = mybir.dt.float16
    AF = mybir.ActivationFunctionType

    nc = bacc.Bacc("TRN2", target_bir_lowering=False, debug=False,
                   num_devices=NCORES)

    # ---- DRAM I/O ----  (tensor data arrives host-cast to fp16; the
    # xbar DMA-transpose path needs a 2-byte dtype anyway)
    mask16_d = nc.dram_tensor("mask16", [R, N], f16, kind="ExternalInput")
    xs16_d = nc.dram_tensor("xs16", [N, FIN], f16, kind="ExternalInput")
    xb16_d = nc.dram_tensor("xb16", [R, FIN], f16, kind="ExternalInput")
    Ws16_d = nc.dram_tensor("ws16", [HF, FIN], f16, kind="ExternalInput")
    sWs16_d = nc.dram_tensor("sws16", [HF, FIN], f16, kind="ExternalInput")
    bias16_d = nc.dram_tensor("bias16", [1, HF], f16, kind="ExternalInput")
    asrc_d = nc.dram_tensor("a_src", [HF], f32, kind="ExternalInput")
    atgt_d = nc.dram_tensor("a_tgt", [HF], f32, kind="ExternalInput")
    out_d = nc.dram_tensor("out", [R, HF], f32, kind="ExternalOutput")
    zscr_d = nc.dram_tensor("zscr", [H, R], f32, kind="Internal")

    from contextlib import ExitStack
    with tile.TileContext(nc) as tc, \
         tc.tile_pool(name="persist", bufs=1) as pp:
        prep_ctx = ExitStack()
        prep = prep_ctx.enter_context(tc.tile_pool(name="prep", bufs=1))

        # ============ phase 0: transposes (DMA) =========================
        # transposed fp16 views via xbar DMA-transpose (HWDGE).  Small
        # transposes go on the scalar HWDGE queue (ACT is idle in prep);
        # the 32 mask transposes alternate between both queues.
        xT = prep.tile([128, KC, N], f16)         # [k_lo, kc, n]
        xbT = prep.tile([128, KC, R], f16)        # [k_lo, kc, i] (own rows)
        WT = prep.tile([128, KC, HF], f16)        # [k_lo, kc, hf]
        sWT = prep.tile([128, KC, HF], f16)
        for kc in range(KC):
            nc.sync.dma_start(out=xT[:, kc, :],
                              in_=xs16_d.ap()[:, bass.ts(kc, 128)],
                              transpose=True)
            nc.sync.dma_start(out=xbT[:, kc, :],
                              in_=xb16_d.ap()[:, bass.ts(kc, 128)],
                              transpose=True)
            nc.sync.dma_start(out=WT[:, kc, :],
                              in_=Ws16_d.ap()[:, bass.ts(kc, 128)],
                              transpose=True)
            nc.sync.dma_start(out=sWT[:, kc, :],
                              in_=sWs16_d.ap()[:, bass.ts(kc, 128)],
                              transpose=True)
        maskT = pp.tile([128, NB, R], f16)      # [j_lo, jb, i]
        njb = NB if "maskt" not in ABL else 1
        for jb in range(njb):
            nc.sync.dma_start(out=maskT[:, jb, :],
                              in_=mask16_d.ap()[:, bass.ts(jb, 128)],
                              transpose=True)

        # untransposed small loads
        Wsb = prep.tile([128, 4, FIN], f16)       # [hf_lo, hfc, k]
        nc.sync.dma_start(out=Wsb[:],
                          in_=Ws16_d.ap().rearrange("(c p) k -> p c k", p=128))
        acol_src = prep.tile([128, 4], f32)       # a_src as [hf_lo, hfc]
        acol_tgt = prep.tile([128, 4], f32)
        nc.sync.dma_start(out=acol_src[:],
                          in_=asrc_d.ap().rearrange("(c p) -> p c", p=128))
        nc.sync.dma_start(out=acol_tgt[:],
                          in_=atgt_d.ap().rearrange("(c p) -> p c", p=128))
        bias16 = prep.tile([1, HF], f16)
        nc.sync.dma_start(out=bias16[:], in_=bias16_d.ap())

        # constants
        ones_row = prep.tile([1, 128], f16)
        nc.vector.memset(ones_row[:], 1.0)
        hsel = prep.tile([128, 2], f16)           # head-selector 0/1 columns
        nc.vector.memset(hsel[:], 0.0)
        nc.vector.memset(hsel[0:64, 0:1], 1.0)
        nc.vector.memset(hsel[64:128, 1:2], 1.0)
        ident = pp.tile([128, 128], f32)
        masks.make_identity(nc, ident[:])

        # ================= phase 1: PE preprocessing ====================
        with tc.tile_pool(name="ps_big", bufs=2, space="PSUM") as psb, \
             tc.tile_pool(name="ps_small", bufs=2, space="PSUM") as pss:

            # wsc_{src,tgt}[hf, k] = W[hf, k] * a[hf]
            wsc_src = prep.tile([128, 4, FIN], f16)
            wsc_tgt = prep.tile([128, 4, FIN], f16)
            for c in range(4):
                nc.vector.tensor_scalar_mul(wsc_src[:, c, :], Wsb[:, c, :],
                                            acol_src[:, c:c + 1])
                nc.vector.tensor_scalar_mul(wsc_tgt[:, c, :], Wsb[:, c, :],
                                            acol_tgt[:, c:c + 1])

            # ws_{src,tgt}[k, h] = sum_f W[(h,f), k] * a[h, f]
            ws_src = pp.tile([128, KC, H], f16)
            ws_tgt = pp.tile([128, KC, H], f16)
            for kc in range(KC):
                for (wsc, ws) in ((wsc_src, ws_src), (wsc_tgt, ws_tgt)):
                    pw = pss.tile([128, H], f32, tag="small")
                    for c in range(4):
                        nc.tensor.matmul(pw[:, 2 * c:2 * c + 2],
                                         wsc[:, c, bass.ts(kc, 128)],
                                         hsel[:],
                                         start=True, stop=True)
                    nc.vector.tensor_copy(ws[:, kc, :], pw[:])

            # projE[n, h, 0:F] = proj, projE[n, h, F] = 1.0
            projE = pp.tile([128, NB, H, F + 1], f16)
            for nb in range(NB):
                ps = psb.tile([128, HF], f32, tag="big")
                for kc in range(KC):
                    nc.tensor.matmul(ps[:], xT[:, kc, bass.ts(nb, 128)],
                                     WT[:, kc, :],
                                     start=(kc == 0), stop=(kc == KC - 1))
                ps_hf = ps[:].rearrange("p (h f) -> p h f", f=F)
                if nb % 2 == 0:
                    nc.scalar.activation(projE[:, nb, :, 0:F], ps_hf, AF.Copy)
                else:
                    nc.vector.tensor_copy(projE[:, nb, :, 0:F], ps_hf)
            nc.vector.memset(projE[:, :, :, F:F + 1], 1.0)

            # s_tgt[n, h] for all n (per-partition operand of the TS pass)
            s_tgt_nh = pp.tile([128, NB, H], f32)
            for nb in range(NB):
                pt = pss.tile([128, H], f32, tag="small")
                for kc in range(KC):
                    nc.tensor.matmul(pt[:], xT[:, kc, bass.ts(nb, 128)],
                                     ws_tgt[:, kc, :],
                                     start=(kc == 0), stop=(kc == KC - 1))
                nc.vector.tensor_copy(s_tgt_nh[:, nb, :], pt[:])

            # s_src rows for the core's own i: [h, i]
            s_src_sb = prep.tile([H, R], f16)
            pss2 = psb.tile([H, R], f32, tag="big")
            for kc in range(KC):
                nc.tensor.matmul(pss2[:], ws_src[:, kc, :], xbT[:, kc, :],
                                 start=(kc == 0), stop=(kc == KC - 1))
            nc.vector.tensor_copy(s_src_sb[:], pss2[:])

            # broadcast s_src rows across partitions: sbc[h][j_lo, i]
            # sel8[:, h, :] is an [8, 128] selector picking row h of s_src_sb
            # sel8[h', h, m] = 1 if h' == h else 0, via affine iota compare
            sel8 = prep.tile([8, H, 128], f16)
            nc.gpsimd.memset(sel8[:], 0.0)
            nc.gpsimd.affine_select(
                out=sel8[:], in_=sel8[:],
                compare_op=mybir.AluOpType.not_equal,
                fill=1.0, base=0,
                # iota = h' * 1 + h * (-1) + m * 0; != 0 -> keep 0, == 0 -> 1
                pattern=[[-1, H], [0, 128]],
                channel_multiplier=1)
            sbc = pp.tile([128, H, R], f16)
            for h in range(H):
                pb = psb.tile([128, R], f32, tag="big")
                nc.tensor.matmul(pb[:], sel8[:, h, :], s_src_sb[:],
                                 start=True, stop=True)
                nc.vector.tensor_copy(sbc[:, h, :], pb[:])

            # skip projection + bias (bias folded as a rank-1 accumulate)
            skipb = pp.tile([128, IC, HF], f32)
            for ic in range(IC):
                pk = psb.tile([128, HF], f32, tag="big")
                for kc in range(KC):
                    nc.tensor.matmul(pk[:], xbT[:, kc, bass.ts(ic, 128)],
                                     sWT[:, kc, :],
                                     start=(kc == 0), stop=False)
                nc.tensor.matmul(pk[:], ones_row[:], bias16[:],
                                 start=False, stop=True)
                nc.vector.tensor_copy(skipb[:, ic, :], pk[:])

        # ================= phase 2: attention main loop =================
        prep_ctx.close()
        oT = pp.tile([F + 1, H, R], f32)        # [f(+Z), h, i]
        with tc.tile_pool(name="ps_agg", bufs=3, space="PSUM") as psa, \
             tc.tile_pool(name="hbuf", bufs=2) as hpool, \
             tc.tile_pool(name="fin", bufs=2) as fpool:
            for h in range(H):
                v = hpool.tile([128, NB, R], f16, tag="v")
                # v = maskT + s_tgt[j]  (per-partition scalar, per j-block)
                if "ts" not in ABL:
                    for jb in range(NB):
                        nc.vector.tensor_scalar_add(v[:, jb, :], maskT[:, jb, :],
                                                    s_tgt_nh[:, jb, h:h + 1])
                else:
                    nc.vector.tensor_copy(v[:, 0, :], maskT[:, 0, :])
                # v += s_src[i]  (one merged TT, broadcast over jb)
                if "tt" not in ABL:
                    nc.vector.tensor_add(
                        v[:], v[:],
                        sbc[:, h:h + 1, :].broadcast_to([128, NB, R]))
                # p = exp(leaky_relu(v))
                if "act" not in ABL:
                    nc.scalar.activation(v[:], v[:], AF.Prelu, alpha=0.2)
                    nc.scalar.activation(v[:], v[:], AF.Exp)
                # aggregate: psum[f, i] += projE[:, jb, h].T @ p[:, jb]
                pa = psa.tile([128, R], f32, tag="agg")
                if "agg" not in ABL:
                    for jb in range(NB):
                        nc.tensor.matmul(pa[0:F + 1, :], projE[:, jb, h, :],
                                         v[:, jb, :],
                                         start=(jb == 0), stop=(jb == NB - 1))
                else:
                    nc.tensor.matmul(pa[0:F + 1, :], projE[:, 0, h, :],
                                     v[:, 0, :], start=True, stop=True)
                nc.vector.tensor_copy(oT[:, h, :], pa[0:F + 1, :])

            # ============= phase 3: normalize, skip, ELU ================
            # shuffle Z into [i_lo, ic, h] layout (tiny partition-scatter),
            # then reciprocal across all 128 partitions (fast)
            recZT = pp.tile([128, IC, H], f32)
            nc.gpsimd.dma_start(out=zscr_d.ap(), in_=oT[F:F + 1, :, :])
            for ic in range(IC):
                nc.gpsimd.dma_start(
                    out=recZT[:, ic, :],
                    in_=zscr_d.ap()[:, bass.ts(ic, 128)].rearrange("h p -> p h"))
            nc.vector.reciprocal(recZT[:], recZT[:])

            out_sb = pp.tile([128, IC, HF], f32)
            for ic in range(IC):
                pT = psa.tile([128, HF], f32, tag="agg")
                for h in range(H):
                    nc.tensor.transpose(pT[:, bass.ts(h, F)],
                                        oT[0:F, h, bass.ts(ic, 128)],
                                        ident[0:F, 0:F])
                y = fpool.tile([128, H, F], f32, tag="y")
                nc.vector.tensor_mul(
                    y[:], pT[:].rearrange("p (h f) -> p h f", f=F),
                    recZT[:, ic, :].unsqueeze(2).broadcast_to([128, H, F]))
                nc.vector.tensor_add(
                    y[:], y[:],
                    skipb[:, ic, :].rearrange("p (h f) -> p h f", f=F))
                # elu(y) = max(y, 0) + min(exp(y) - 1, 0)
                q = fpool.tile([128, H, F], f32, tag="q")
                nc.scalar.activation(q[:], y[:], AF.Exp)
                nc.vector.tensor_scalar(q[:], q[:], 1.0, 0.0,
                                        op.subtract, op.min)
                nc.vector.tensor_scalar(y[:], y[:], 0.0, None, op.max)
                nc.vector.tensor_add(
                    out_sb[:, ic, :].rearrange("p (h f) -> p h f", f=F),
                    y[:], q[:])
            nc.sync.dma_start(
                out=out_d.ap().rearrange("(c p) f -> p c f", p=128),
                in_=out_sb[:])

    nc.compile()
    return nc


def _get_nc():
    if "nc" not in _cache:
        _cache["nc"] = _build()
    return _cache["nc"]


def kernel(x, connectivity_mask, W, a_src, a_tgt, skip_W, bias):
    from concourse.bass_utils import run_bass_kernel_spmd

    x16 = np.ascontiguousarray(np.asarray(x, dtype=np.float16))
    cm = np.asarray(connectivity_mask, dtype=np.float32)
    # clip so -1e9 doesn't overflow fp16 (-6e4 still drives exp to 0)
    cm16 = np.clip(cm, -60000.0, None).astype(np.float16)
    W16 = np.ascontiguousarray(np.asarray(W, dtype=np.float16))
    sW16 = np.ascontiguousarray(np.asarray(skip_W, dtype=np.float16))
    b16 = np.ascontiguousarray(
        np.asarray(bias, dtype=np.float16).reshape(1, HF))
    asrc = np.ascontiguousarray(np.asarray(a_src, dtype=np.float32).reshape(HF))
    atgt = np.ascontiguousarray(np.asarray(a_tgt, dtype=np.float32).reshape(HF))

    in_maps = []
    for c in range(NCORES):
        in_maps.append({
            "xs16": x16,
            "xb16": np.ascontiguousarray(x16[c * R:(c + 1) * R]),
            "mask16": np.ascontiguousarray(cm16[c * R:(c + 1) * R]),
            "ws16": W16,
            "sws16": sW16,
            "bias16": b16,
            "a_src": asrc,
            "a_tgt": atgt,
        })

    nc = _get_nc()
    res = run_bass_kernel_spmd(nc, in_maps, core_ids=list(range(NCORES)))
    return np.concatenate([r["out"] for r in res.results], axis=0)
